# revision 7
# baseline (speedup 1.0000x reference)
"""Data-parallel FFLayer kernel for 8 TRN2 NeuronCores (Bass/Tile).

Computes  out = relu( (x / (||x||_2_row + 1e-4)) @ W.T + b )  for
x [16384, 2048], W [2048, 2048], b [2048], all float32.

Sharding (data-parallel): x is split along batch into 8 shards of
[2048, 2048]; W and b are replicated.  W is passed to the device
pre-transposed (W.T, a host-side layout prep) so the contraction dim
lands on SBUF partitions for both matmul operands.

Per-core pipeline:
  * W.T streams in once as fp32 and is cast to bf16 on GPSIMD; the 16
    bf16 k-slices [128, 2048] stay pinned in SBUF.
  * For each of 16 row-tiles [128, 2048] of x:
      1. DMA x tile in (fp32).
      2. ScalarE Square activation with accum_out -> row sum-of-squares
         (fp32, full precision; squared values dumped into the xT
         buffer which is overwritten later anyway).
      3. sqrt (+eps, reciprocal) -> per-row scale s [128,1].
      4. GPSIMD applies the scale and casts to bf16 in one
         tensor_scalar_mul (per-partition scalar) -> xb = x_dir bf16.
      5. PE transpose-mode of each 128x128 bf16 block (vs identity;
         transpose-mode ignores the moving operand's values, verified
         on HW) -> x_dir.T in PSUM, copied to SBUF (ScalarE/DVE).
      6. Main bf16 matmul: 16 k-tiles accumulate into PSUM, 512
         output cols per matmul.
      7. DVE adds bias (fp32) on PSUM->SBUF eviction, ScalarE ReLU,
         DMA out (fp32).
"""

import numpy as np

B, IN, OUT, NCORES = 16384, 2048, 2048, 8
BS = B // NCORES  # batch rows per core
P = 128
NB = BS // P  # b-tiles per core
NK = IN // P  # k-tiles
EPS = 1e-4

_NC_CACHE = {}


def _build_nc():
    import concourse.mybir as mybir
    import concourse.tile as tile
    from concourse import bacc
    from concourse.masks import make_identity

    f32 = mybir.dt.float32
    bf16 = mybir.dt.bfloat16
    AF = mybir.ActivationFunctionType

    nc = bacc.Bacc()
    x_d = nc.declare_dram_parameter("x", [BS, IN], f32, isOutput=False)
    wt_d = nc.declare_dram_parameter("wt", [IN, OUT], f32, isOutput=False)
    b_d = nc.declare_dram_parameter("bias", [1, OUT], f32, isOutput=False)
    out_d = nc.declare_dram_parameter("out", [BS, OUT], f32, isOutput=True)

    with tile.TileContext(nc) as tc:
        with (
            tc.tile_pool(name="wtf", bufs=2) as wtf,
            tc.tile_pool(name="wtb", bufs=1) as wtb,
            tc.tile_pool(name="consts", bufs=1) as consts,
            tc.tile_pool(name="xin", bufs=2) as xin,
            tc.tile_pool(name="xbp", bufs=2) as xbp,
            tc.tile_pool(name="xt", bufs=2) as xtp,
            tc.tile_pool(name="outp", bufs=3) as outp,
            tc.tile_pool(name="small", bufs=6) as small,
            tc.tile_pool(name="pt", bufs=2, space="PSUM") as ptp,
            tc.tile_pool(name="po", bufs=2, space="PSUM") as pop,
        ):
            bias_sb = consts.tile([P, OUT], f32)
            nc.sync.dma_start(bias_sb, b_d[:].to_broadcast((P, OUT)))

            ident = consts.tile([P, P], bf16)
            make_identity(nc, ident)

            wt_sb = []
            for ko in range(NK):
                tf = wtf.tile([P, OUT], f32)
                nc.sync.dma_start(tf, wt_d[ko * P : (ko + 1) * P, :])
                tb = wtb.tile([P, OUT], bf16, tag=f"wt{ko}")
                nc.gpsimd.tensor_copy(tb, tf)
                wt_sb.append(tb)

            for bt in range(NB):
                x_t = xin.tile([P, IN], f32)
                nc.sync.dma_start(x_t, x_d[bt * P : (bt + 1) * P, :])

                xT = xtp.tile([P, NK, P], bf16)
                nsq = small.tile([P, 1], f32)
                nc.scalar.activation(
                    out=xT.rearrange("p a b -> p (a b)"),
                    in_=x_t,
                    func=AF.Square,
                    accum_out=nsq,
                )
                nrm = small.tile([P, 1], f32)
                nc.scalar.activation(out=nrm, in_=nsq, func=AF.Sqrt)
                nc.vector.tensor_scalar_add(nrm, nrm, EPS)
                s = small.tile([P, 1], f32)
                nc.vector.reciprocal(s, nrm)

                # scale + cast to bf16 in one op: xb = x_dir
                xb = xbp.tile([P, IN], bf16)
                nc.gpsimd.tensor_scalar_mul(xb, x_t, s)

                for ko in range(NK):
                    pt = ptp.tile([P, P], bf16)
                    nc.tensor.transpose(pt, xb[:, ko * P : (ko + 1) * P], ident)
                    if ko % 2 == 0:
                        nc.scalar.copy(xT[:, ko, :], pt)
                    else:
                        nc.vector.tensor_copy(xT[:, ko, :], pt)

                for h in range(2):
                    ps = pop.tile([P, 1024], f32)
                    for ko in range(NK):
                        for n2 in range(2):
                            c0 = h * 1024 + n2 * 512
                            nc.tensor.matmul(
                                ps[:, n2 * 512 : (n2 + 1) * 512],
                                lhsT=xT[:, ko, :],
                                rhs=wt_sb[ko][:, c0 : c0 + 512],
                                start=(ko == 0),
                                stop=(ko == NK - 1),
                            )
                    o_sb = outp.tile([P, 1024], f32)
                    for n2 in range(2):
                        lo = n2 * 512
                        nc.vector.tensor_add(
                            o_sb[:, lo : lo + 512],
                            ps[:, lo : lo + 512],
                            bias_sb[:, h * 1024 + lo : h * 1024 + lo + 512],
                        )
                        nc.scalar.activation(
                            o_sb[:, lo : lo + 512],
                            o_sb[:, lo : lo + 512],
                            AF.Relu,
                        )
                    nc.sync.dma_start(
                        out_d[bt * P : (bt + 1) * P, h * 1024 : (h + 1) * 1024],
                        o_sb,
                    )

    nc.compile()
    return nc


def _get_nc():
    if "nc" not in _NC_CACHE:
        _NC_CACHE["nc"] = _build_nc()
    return _NC_CACHE["nc"]


def _make_in_maps(x, W, b):
    x = np.ascontiguousarray(np.asarray(x, dtype=np.float32))
    W = np.asarray(W, dtype=np.float32)
    b = np.asarray(b, dtype=np.float32)
    wt = np.ascontiguousarray(W.T)
    bias = np.ascontiguousarray(b.reshape(1, OUT))
    return [
        {
            "x": np.ascontiguousarray(x[i * BS : (i + 1) * BS]),
            "wt": wt,
            "bias": bias,
        }
        for i in range(NCORES)
    ]


def _run(x, W, b, trace=False):
    from concourse.bass_utils import run_bass_kernel_spmd

    nc = _get_nc()
    res = run_bass_kernel_spmd(
        nc, _make_in_maps(x, W, b), core_ids=list(range(NCORES)), trace=trace
    )
    out = np.concatenate(
        [np.asarray(res.results[i]["out"]) for i in range(NCORES)], axis=0
    )
    return out, res


def kernel(**inputs):
    out, _ = _run(inputs["x"], inputs["W"], inputs["b"])
    return out


def run_profiled(**inputs):
    out, res = _run(inputs["x"], inputs["W"], inputs["b"], trace=True)
    return out, res


# revision 10
# speedup vs baseline: 1.7391x; 1.7391x over previous
"""Data-parallel FFLayer kernel for 8 TRN2 NeuronCores (Bass/Tile).

Computes  out = relu( (x / (||x||_2_row + 1e-4)) @ W.T + b )  for
x [16384, 2048], W [2048, 2048], b [2048], all float32.

Sharding (data-parallel): x is split along batch into 8 shards of
[2048, 2048]; W and b are replicated.  W is passed to the device
pre-transposed (W.T, a host-side layout prep) so the contraction dim
lands on SBUF partitions for both matmul operands.

Per-core pipeline:
  * W.T streams in once as fp32 and is cast to bf16 on GPSIMD; the 16
    bf16 k-slices [128, 2048] stay pinned in SBUF.
  * For each of 16 row-tiles [128, 2048] of x:
      1. DMA x tile in (fp32).
      2. ScalarE Square activation with accum_out -> row sum-of-squares
         (fp32, full precision; squared values dumped into the xT
         buffer which is overwritten later anyway).
      3. sqrt (+eps, reciprocal) -> per-row scale s [128,1].
      4. DVE applies the scale and casts to bf16 in one
         tensor_scalar_mul (per-partition scalar) -> xb = x_dir bf16.
         (GPSIMD is dramatically slower for these wide elementwise ops
         and port-blocks DVE -- measured 85% GpSimd busy when used.)
      5. PE transpose-mode of each 128x128 bf16 block (vs identity;
         transpose-mode ignores the moving operand's values, verified
         on HW) -> x_dir.T in PSUM, copied to SBUF (ScalarE/DVE).
      6. Main bf16 matmul: 16 k-tiles accumulate into PSUM, 512
         output cols per matmul.
      7. DVE adds bias (fp32) on PSUM->SBUF eviction, ScalarE ReLU,
         DMA out (fp32).
"""

import numpy as np

B, IN, OUT, NCORES = 16384, 2048, 2048, 8
BS = B // NCORES  # batch rows per core
P = 128
NB = BS // P  # b-tiles per core
NK = IN // P  # k-tiles
EPS = 1e-4

_NC_CACHE = {}


def _build_nc():
    import concourse.mybir as mybir
    import concourse.tile as tile
    from concourse import bacc
    from concourse.masks import make_identity

    f32 = mybir.dt.float32
    bf16 = mybir.dt.bfloat16
    AF = mybir.ActivationFunctionType

    nc = bacc.Bacc()
    x_d = nc.declare_dram_parameter("x", [BS, IN], f32, isOutput=False)
    wt_d = nc.declare_dram_parameter("wt", [IN, OUT], f32, isOutput=False)
    b_d = nc.declare_dram_parameter("bias", [1, OUT], f32, isOutput=False)
    out_d = nc.declare_dram_parameter("out", [BS, OUT], f32, isOutput=True)

    with tile.TileContext(nc) as tc:
        with (
            tc.tile_pool(name="wtf", bufs=2) as wtf,
            tc.tile_pool(name="wtb", bufs=1) as wtb,
            tc.tile_pool(name="consts", bufs=1) as consts,
            tc.tile_pool(name="xin", bufs=2) as xin,
            tc.tile_pool(name="xbp", bufs=2) as xbp,
            tc.tile_pool(name="xt", bufs=2) as xtp,
            tc.tile_pool(name="outp", bufs=3) as outp,
            tc.tile_pool(name="small", bufs=6) as small,
            tc.tile_pool(name="pt", bufs=2, space="PSUM") as ptp,
            tc.tile_pool(name="po", bufs=2, space="PSUM") as pop,
        ):
            bias_sb = consts.tile([P, OUT], f32)
            nc.sync.dma_start(bias_sb, b_d[:].to_broadcast((P, OUT)))

            ident = consts.tile([P, P], bf16)
            make_identity(nc, ident)

            wt_sb = []
            for ko in range(NK):
                tf = wtf.tile([P, OUT], f32)
                nc.sync.dma_start(tf, wt_d[ko * P : (ko + 1) * P, :])
                tb = wtb.tile([P, OUT], bf16, tag=f"wt{ko}")
                nc.scalar.copy(tb, tf)
                wt_sb.append(tb)

            for bt in range(NB):
                x_t = xin.tile([P, IN], f32)
                nc.sync.dma_start(x_t, x_d[bt * P : (bt + 1) * P, :])

                xT = xtp.tile([P, NK, P], bf16)
                nsq = small.tile([P, 1], f32)
                nc.scalar.activation(
                    out=xT.rearrange("p a b -> p (a b)"),
                    in_=x_t,
                    func=AF.Square,
                    accum_out=nsq,
                )
                nrm = small.tile([P, 1], f32)
                nc.scalar.activation(out=nrm, in_=nsq, func=AF.Sqrt)
                nc.vector.tensor_scalar_add(nrm, nrm, EPS)
                s = small.tile([P, 1], f32)
                nc.vector.reciprocal(s, nrm)

                # scale + cast to bf16 in one op: xb = x_dir
                xb = xbp.tile([P, IN], bf16)
                nc.vector.tensor_scalar_mul(xb, x_t, s)

                for ko in range(NK):
                    pt = ptp.tile([P, P], bf16)
                    nc.tensor.transpose(pt, xb[:, ko * P : (ko + 1) * P], ident)
                    if ko % 2 == 0:
                        nc.scalar.copy(xT[:, ko, :], pt)
                    else:
                        nc.vector.tensor_copy(xT[:, ko, :], pt)

                for h in range(2):
                    ps = pop.tile([P, 1024], f32)
                    for ko in range(NK):
                        for n2 in range(2):
                            c0 = h * 1024 + n2 * 512
                            nc.tensor.matmul(
                                ps[:, n2 * 512 : (n2 + 1) * 512],
                                lhsT=xT[:, ko, :],
                                rhs=wt_sb[ko][:, c0 : c0 + 512],
                                start=(ko == 0),
                                stop=(ko == NK - 1),
                            )
                    o_sb = outp.tile([P, 1024], f32)
                    for n2 in range(2):
                        lo = n2 * 512
                        nc.vector.tensor_add(
                            o_sb[:, lo : lo + 512],
                            ps[:, lo : lo + 512],
                            bias_sb[:, h * 1024 + lo : h * 1024 + lo + 512],
                        )
                        nc.scalar.activation(
                            o_sb[:, lo : lo + 512],
                            o_sb[:, lo : lo + 512],
                            AF.Relu,
                        )
                    nc.sync.dma_start(
                        out_d[bt * P : (bt + 1) * P, h * 1024 : (h + 1) * 1024],
                        o_sb,
                    )

    nc.compile()
    return nc


def _get_nc():
    if "nc" not in _NC_CACHE:
        _NC_CACHE["nc"] = _build_nc()
    return _NC_CACHE["nc"]


def _make_in_maps(x, W, b):
    x = np.ascontiguousarray(np.asarray(x, dtype=np.float32))
    W = np.asarray(W, dtype=np.float32)
    b = np.asarray(b, dtype=np.float32)
    wt = np.ascontiguousarray(W.T)
    bias = np.ascontiguousarray(b.reshape(1, OUT))
    return [
        {
            "x": np.ascontiguousarray(x[i * BS : (i + 1) * BS]),
            "wt": wt,
            "bias": bias,
        }
        for i in range(NCORES)
    ]


def _run(x, W, b, trace=False):
    from concourse.bass_utils import run_bass_kernel_spmd

    nc = _get_nc()
    res = run_bass_kernel_spmd(
        nc, _make_in_maps(x, W, b), core_ids=list(range(NCORES)), trace=trace
    )
    out = np.concatenate(
        [np.asarray(res.results[i]["out"]) for i in range(NCORES)], axis=0
    )
    return out, res


def kernel(**inputs):
    out, _ = _run(inputs["x"], inputs["W"], inputs["b"])
    return out


def run_profiled(**inputs):
    out, res = _run(inputs["x"], inputs["W"], inputs["b"], trace=True)
    return out, res
